# revision 10
# baseline (speedup 1.0000x reference)
"""LatticeLSTM (BiLSTM w/ word cells) Trainium2 kernel, v2.

Sharding: time-sharded, 2 windows per core. The 512-step scan splits
into 16 windows of C=32 owned steps; core k runs windows {2k, 2k+1}
simultaneously as extra lanes. Each window computes T = W + C = 48
local steps, where the first W=16 steps warm up the state from zero
(coupled forget gate contracts ~0.5/step, so warm-start error is
~2^-14 by the first owned step). Window 0 starts at global step 0
with no warm-up (its last W steps are wasted instead). No
collectives.

Device layout: feature/gate index on SBUF partitions, lanes on the
free dim. Lanes per direction L = 64 (2 windows x 32 batch). The fw
and bw recurrences are fully independent streams with separate PSUM
banks, weights, and work tiles, emitted interleaved so their serial
elementwise chains pipeline across the Act/DVE/GpSimd engines.

Per-step PSUM banks (per direction):
  pg [128, 8, 64]: pre-gates, chunk order i(2) o(2) alpha(2) g(2) so
     one Sigmoid covers chunks 0:6 and one Tanh covers 6:8.
  pw [128, 6, 64]: word gates iw(2) fw(2) gw(2).
Biases are injected by a [K=chunks, M=128] x [K, chunks*lanes]
selector matmul (start=True clears the bank), then x-projections and
h-matmuls accumulate on top.

The three mask-lerps (merge cell, merge input gate, pending-word
update) use in-place copy_predicated instead of 3-op lerps. Masks
depend only on the integer word-lattice inputs and are precomputed on
host, as is the skip_input reversal; embedding rows are gathered on
host (bf16).
"""

import numpy as np
import ml_dtypes

import concourse.bass as bass
import concourse.bacc as bacc
import concourse.tile as tile
from concourse import mybir
from concourse.bass_utils import run_bass_kernel_spmd

B, S, E, H, V, L = 32, 512, 128, 256, 21128, 32
NCORES = 8
G = 4                       # windows per core
C = 16                      # owned steps per window
WARM = 12
T = C + WARM                # 28 local steps
LD = 32 * G                 # lanes per direction (128)
NIDX = T * LD               # gathered rows per table per direction
NT = 4                      # tag matmul: steps per N-chunk

f32 = mybir.dt.float32
bf16 = mybir.dt.bfloat16
u8 = mybir.dt.uint8
f16 = mybir.dt.float16
i16 = mybir.dt.int16
Sig = mybir.ActivationFunctionType.Sigmoid
Tanh = mybir.ActivationFunctionType.Tanh

bf = ml_dtypes.bfloat16

_CACHE = {}

DIRS = ("f", "b")
# whh tile chunk mi -> pg chunk (i,i,o,o,g,g -> 0,1,2,3,6,7)
PG_HCH = (0, 1, 2, 3, 6, 7)


def _flat2(ap3):
    """[128, a, b] AP with contiguous free dims -> [128, a*b]."""
    p, (sa, ca), (sb, cb) = ap3.ap
    assert sa == sb * cb, f"non-contiguous free dims: {ap3.ap}"
    return bass.AP(tensor=ap3.tensor, offset=ap3.offset,
                   ap=[p, [sb, ca * cb]])


def _build_bass():
    nc = bacc.Bacc(None, target_bir_lowering=False)

    def inp(name, shape, dtype):
        return nc.declare_dram_parameter(name, list(shape), dtype, isOutput=False)

    xT_d = {d: inp(f"x_T_{d}", [128, NIDX], bf16) for d in DIRS}
    weT_d = {d: inp(f"we_T_{d}", [128, NIDX], bf16) for d in DIRS}
    # x-side combined weight (lhsT): chunks i,i,o,o,a,a,g,g
    xw_d = {d: inp(f"xw_{d}", [E, 8, 128], bf16) for d in DIRS}
    wwih_d = {d: inp(f"wwih_{d}", [E, 6, 128], bf16) for d in DIRS}
    # h-side weight tiles: [128 K-rows, kc, m, 128]
    whh_d = {d: inp(f"whh_{d}", [128, 2, 6, 128], bf16) for d in DIRS}
    wwhh_d = {d: inp(f"wwhh_{d}", [128, 2, 6, 128], bf16) for d in DIRS}
    awhh_d = {d: inp(f"awhh_{d}", [128, 2, 2, 128], bf16) for d in DIRS}
    # bias lhsT rows match pg/pw chunk order, split per psum bank
    bgA_d = {d: inp(f"biasgA_{d}", [4, 128], bf16) for d in DIRS}
    bgB_d = {d: inp(f"biasgB_{d}", [4, 128], bf16) for d in DIRS}
    bwA_d = {d: inp(f"biaswA_{d}", [4, 128], bf16) for d in DIRS}
    bwB_d = {d: inp(f"biaswB_{d}", [2, 128], bf16) for d in DIRS}
    sel4_d = inp("sel4", [4, 4 * LD], bf16)   # sel[k, c*LD+l] = (c==k)
    sel2_d = inp("sel2", [2, 2 * LD], bf16)
    maskm_d = {d: inp(f"mask_m_{d}", [T, 2 * LD], u8) for d in DIRS}
    maskw_d = {d: inp(f"mask_w_{d}", [T, 2 * LD], u8) for d in DIRS}
    tagw_d = inp("tagw", [128, 2, 2, 32], bf16)   # [K-row, dir, kc, label]

    out_d = nc.declare_dram_parameter("out_tags", [2, 32, T * LD], f32,
                                      isOutput=True)

    with tile.TileContext(nc) as tc:
        with (
            tc.tile_pool(name="const", bufs=1) as cpool,
            tc.tile_pool(name="state", bufs=1) as spool,
            tc.tile_pool(name="work", bufs=2) as wpool,
            tc.tile_pool(name="outp", bufs=4) as opool,
            tc.tile_pool(name="psumGf", bufs=1, space="PSUM") as psGf,
            tc.tile_pool(name="psumGb", bufs=1, space="PSUM") as psGb,
            tc.tile_pool(name="psumWf", bufs=1, space="PSUM") as psWf,
            tc.tile_pool(name="psumWb", bufs=1, space="PSUM") as psWb,
        ):
            psG = {"f": psGf, "b": psGb}
            psW = {"f": psWf, "b": psWb}

            # ---- load constants ----
            def load(dram, shape, dtype, tag):
                t_ = cpool.tile(list(shape), dtype, tag=tag, name=tag)
                nc.sync.dma_start(out=t_[...], in_=dram[...])
                return t_

            xw = {d: load(xw_d[d], [E, 8, 128], bf16, f"xw{d}") for d in DIRS}
            wwih = {d: load(wwih_d[d], [E, 6, 128], bf16, f"wwih{d}") for d in DIRS}
            whh = {d: load(whh_d[d], [128, 2, 6, 128], bf16, f"whh{d}") for d in DIRS}
            wwhh = {d: load(wwhh_d[d], [128, 2, 6, 128], bf16, f"wwhh{d}")
                    for d in DIRS}
            awhh = {d: load(awhh_d[d], [128, 2, 2, 128], bf16, f"awhh{d}")
                    for d in DIRS}
            bgA = {d: load(bgA_d[d], [4, 128], bf16, f"bgA{d}") for d in DIRS}
            bgB = {d: load(bgB_d[d], [4, 128], bf16, f"bgB{d}") for d in DIRS}
            bwA = {d: load(bwA_d[d], [4, 128], bf16, f"bwA{d}") for d in DIRS}
            bwB = {d: load(bwB_d[d], [2, 128], bf16, f"bwB{d}") for d in DIRS}
            sel4 = load(sel4_d, [4, 4 * LD], bf16, "sel4")
            sel2 = load(sel2_d, [2, 2 * LD], bf16, "sel2")
            tagw = load(tagw_d, [128, 2, 2, 32], bf16, "tagw")
            x_T = {d: load(xT_d[d], [128, NIDX], bf16, f"xT{d}") for d in DIRS}
            we_T = {d: load(weT_d[d], [128, NIDX], bf16, f"weT{d}") for d in DIRS}

            # masks broadcast to all 128 partitions
            maskm, maskw = {}, {}
            for d in DIRS:
                maskm[d] = cpool.tile([128, T, 2 * LD], u8, tag=f"maskm{d}",
                                      name=f"maskm{d}")
                maskw[d] = cpool.tile([128, T, 2 * LD], u8, tag=f"maskw{d}",
                                      name=f"maskw{d}")
                for md, mt in ((maskm_d[d], maskm[d]), (maskw_d[d], maskw[d])):
                    src = md[...]
                    bsrc = bass.AP(tensor=src.tensor, offset=src.offset,
                                   ap=[[0, 128]] + list(src.ap))
                    nc.sync.dma_start(out=mt[...], in_=bsrc)

            # absorb the mask-DMA completion wait on DVE's vector clock here:
            # copy_predicated (3-AP ISA struct) has only ONE sync-wait slot.
            mwarm = cpool.tile([128, 2 * LD], u8, tag="mwarm", name="mwarm")
            for d in DIRS:
                nc.vector.tensor_copy(mwarm[...], maskm[d][:, 0, :])
                nc.vector.tensor_copy(mwarm[...], maskw[d][:, 0, :])

            # ---- states ----
            h_hist, c_st, pc_st, pc_bf = {}, {}, {}, {}
            for d in DIRS:
                h_hist[d] = spool.tile([128, T + 1, 2, LD], bf16,
                                       tag=f"hh{d}", name=f"hh{d}")
                c_st[d] = spool.tile([128, 2, LD], f16, tag=f"c{d}", name=f"c{d}")
                pc_st[d] = spool.tile([128, 2, LD], f16, tag=f"pc{d}",
                                      name=f"pc{d}")
                pc_bf[d] = spool.tile([128, 2, LD], bf16, tag=f"pcb{d}",
                                      name=f"pcb{d}")
                nc.vector.memset(h_hist[d][:, 0, :, :], 0.0)
                nc.vector.memset(c_st[d][...], 0.0)
                nc.vector.memset(pc_st[d][...], 0.0)
                nc.vector.memset(pc_bf[d][...], 0.0)

            def xcol(tile_, t):
                return tile_[:, t * LD: (t + 1) * LD]

            # pg spans 2 psum banks (chunks 0-3 / 4-7); accumulation groups
            # are per bank, so each bank gets its own start and its last
            # writer carries stop (bank A ends in emit_pg_h, bank B in alpha).
            def emit_pg_inject(d, pg, t):
                """bias + x-side products for step t into pg (opens groups)."""
                nc.tensor.matmul(pg[:, 0:4, :], bgA[d][...], sel4[...],
                                 start=True, stop=False)
                nc.tensor.matmul(pg[:, 4:8, :], bgB[d][...], sel4[...],
                                 start=True, stop=False)
                for m in range(8):
                    nc.tensor.matmul(pg[:, m:m + 1, :], xw[d][:, m, :],
                                     xcol(x_T[d], t), start=False, stop=False)

            def emit_pg_h(d, pg, t):
                """pre-h into pg for step t (reads h_{t-1} = slot t)."""
                for kc in range(2):
                    for mi in range(6):
                        ch = PG_HCH[mi]
                        nc.tensor.matmul(pg[:, ch:ch + 1, :],
                                         whh[d][:, kc, mi, :],
                                         h_hist[d][:, t, kc, :],
                                         start=False,
                                         stop=(kc == 1 and ch == 3))

            def emit_pg_alpha(d, pg):
                """alpha-h (pc) into pg; closes bank B's group."""
                n = 0
                for kc in range(2):
                    for m in range(2):
                        n += 1
                        nc.tensor.matmul(pg[:, 4 + m:5 + m, :],
                                         awhh[d][:, kc, m, :],
                                         pc_bf[d][:, kc, :],
                                         start=False, stop=(n == 4))

            def emit_pw(d, pw, t):
                nc.tensor.matmul(pw[:, 0:4, :], bwA[d][...], sel4[...],
                                 start=True, stop=False)
                nc.tensor.matmul(pw[:, 4:6, :], bwB[d][...], sel2[...],
                                 start=True, stop=False)
                for m in range(6):
                    nc.tensor.matmul(pw[:, m:m + 1, :], wwih[d][:, m, :],
                                     xcol(we_T[d], t), start=False, stop=False)

            def emit_pw_h(d, pw, t):
                for kc in range(2):
                    for m in range(6):
                        nc.tensor.matmul(pw[:, m:m + 1, :],
                                         wwhh[d][:, kc, m, :],
                                         h_hist[d][:, t + 1, kc, :],
                                         start=False,
                                         stop=(kc == 1 and m in (3, 5)))

            def wt(shape, tg):
                return wpool.tile(shape, f16, tag=tg, name=tg)

            # prologue: pg for step 0 (h_{-1}=0, pc=0 tiles)
            pg, pw = {}, {}
            for d in DIRS:
                pg[d] = psG[d].tile([128, 8, LD], f32, tag=f"pg{d}",
                                    name=f"pg{d}")
                emit_pg_inject(d, pg[d], 0)
                emit_pg_h(d, pg[d], 0)
                emit_pg_alpha(d, pg[d])

            def h1(d, t):
                """V1: gates -> c_new, h_t; plus pw inject + pw_h matmuls."""
                pw[d] = psW[d].tile([128, 6, LD], f32, tag=f"pw{d}",
                                    name=f"pw{d}")
                emit_pw(d, pw[d], t)
                ga = wt([128, 6, LD], f"ga{d}")
                nc.scalar.activation(ga[...], pg[d][:, 0:6, :], Sig)
                gt_ = wt([128, 2, LD], f"gt{d}")
                nc.scalar.activation(gt_[...], pg[d][:, 6:8, :], Tanh)
                spre = wt([128, 2, LD], f"spre{d}")
                nc.vector.tensor_sub(spre[...], ga[:, 0:2, :], ga[:, 4:6, :])
                s_ = wt([128, 2, LD], f"s{d}")
                nc.scalar.activation(s_[...], spre[...], Sig)
                m2 = maskm[d][:, t, :]
                nc.vector.copy_predicated(_flat2(ga[:, 0:2, :]), m2,
                                          _flat2(s_[...]))
                nc.vector.copy_predicated(_flat2(c_st[d][...]), m2,
                                          _flat2(pc_st[d][...]))
                d2 = wt([128, 2, LD], f"d2{d}")
                nc.gpsimd.tensor_sub(d2[...], gt_[...], c_st[d][...])
                ed = wt([128, 2, LD], f"ed{d}")
                nc.vector.tensor_mul(ed[...], ga[:, 0:2, :], d2[...])
                nc.vector.tensor_add(c_st[d][...], c_st[d][...], ed[...])
                tc_ = wt([128, 2, LD], f"tc{d}")
                nc.scalar.activation(tc_[...], c_st[d][...], Tanh)
                nc.vector.tensor_mul(h_hist[d][:, t + 1, :, :],
                                     ga[:, 2:4, :], tc_[...])
                emit_pw_h(d, pw[d], t)

            def h2(d, t):
                """V2: word cell -> pc; plus next step's pg group."""
                pgn = None
                if t < T - 1:
                    pgn = psG[d].tile([128, 8, LD], f32, tag=f"pg{d}",
                                      name=f"pg{d}")
                    emit_pg_inject(d, pgn, t + 1)
                gv = wt([128, 4, LD], f"gv{d}")
                nc.scalar.activation(gv[...], pw[d][:, 0:4, :], Sig)
                gwt_ = wt([128, 2, LD], f"gwt{d}")
                nc.scalar.activation(gwt_[...], pw[d][:, 4:6, :], Tanh)
                t2 = wt([128, 2, LD], f"t2{d}")
                nc.gpsimd.tensor_mul(t2[...], gv[:, 0:2, :], gwt_[...])
                t1 = wt([128, 2, LD], f"t1{d}")
                nc.vector.tensor_mul(t1[...], gv[:, 2:4, :], c_st[d][...])
                cw = wt([128, 2, LD], f"cw{d}")
                nc.vector.tensor_add(cw[...], t1[...], t2[...])
                w2 = maskw[d][:, t, :]
                nc.vector.copy_predicated(_flat2(pc_st[d][...]), w2,
                                          _flat2(cw[...]))
                nc.vector.tensor_copy(pc_bf[d][...], pc_st[d][...])
                if t < T - 1:
                    emit_pg_h(d, pgn, t + 1)
                    emit_pg_alpha(d, pgn)
                    pg[d] = pgn

            # software-pipeline the two independent streams half a step
            # apart so their serial chains interleave across engines.
            h1("f", 0)
            for t in range(T):
                h2("f", t)
                h1("b", t)
                if t < T - 1:
                    h1("f", t + 1)
                h2("b", t)

            # ---- tag projection: out[d, label, tau*LD+lane] ----
            for di, d in enumerate(DIRS):
                for n in range(T // NT):
                    pt = psW[d].tile([32, NT, LD], f32, tag=f"pw{d}",
                                     name="pt")
                    for kc in range(2):
                        nc.tensor.matmul(
                            pt[...], tagw[:, di, kc, :],
                            h_hist[d][:, 1 + n * NT: 1 + (n + 1) * NT, kc, :],
                            start=(kc == 0), stop=(kc == 1))
                    ob = opool.tile([32, NT, LD], f32, tag="ob", name="ob")
                    nc.vector.tensor_copy(ob[...], pt[...])
                    nc.sync.dma_start(
                        out=out_d[di, :, n * NT * LD: (n + 1) * NT * LD],
                        in_=ob[...])

    nc.compile()
    return nc


# ------------------------- host side -------------------------

def _window_start(w):
    return 0 if w == 0 else C * w - WARM


def _window_t0(w):
    return 0 if w == 0 else WARM


def _masks_for_window(wlen_win):
    """wlen_win [32, T] int -> merge mask m [T,32], has-word hw [T,32] f32,
    replicating the truncated-from-zero pcnt/pvalid recurrence."""
    n = wlen_win.shape[0]
    pcnt = np.full((n,), -1, np.int64)
    pvalid = np.zeros((n,), bool)
    m = np.zeros((T, n), np.float32)
    hw = np.zeros((T, n), np.float32)
    for t in range(T):
        mg = pvalid & (pcnt == 0)
        m[t] = mg.astype(np.float32)
        pvalid = pvalid & ~mg
        pcnt = pcnt - 1
        w = wlen_win[:, t] >= 2
        hw[t] = w.astype(np.float32)
        pcnt = np.where(w, wlen_win[:, t] - 1, pcnt)
        pvalid = pvalid | w
    return m, hw


def _weight_tiles(Wx, Whh, aWx, aWhh, wWx, wWhh, b, ab, wb):
    r = {}
    w6 = Wx.reshape(E, 6, 128)
    aw2 = aWx.reshape(E, 2, 128)
    r["xw"] = np.ascontiguousarray(
        np.concatenate([w6[:, 0:4], aw2, w6[:, 4:6]], axis=1)).astype(bf)
    r["wwih"] = np.ascontiguousarray(wWx.reshape(E, 6, 128)).astype(bf)
    r["whh"] = np.ascontiguousarray(
        Whh.reshape(2, 128, 6, 128).transpose(1, 0, 2, 3)).astype(bf)
    r["wwhh"] = np.ascontiguousarray(
        wWhh.reshape(2, 128, 6, 128).transpose(1, 0, 2, 3)).astype(bf)
    r["awhh"] = np.ascontiguousarray(
        aWhh.reshape(2, 128, 2, 128).transpose(1, 0, 2, 3)).astype(bf)
    b6 = b.reshape(6, 128)
    bgv = np.zeros((8, 128), np.float32)
    bgv[0:4] = b6[0:4]
    bgv[4:6] = ab.reshape(2, 128)
    bgv[6:8] = b6[4:6]
    r["biasgA"] = bgv[0:4].astype(bf)
    r["biasgB"] = bgv[4:8].astype(bf)
    wb6 = wb.reshape(6, 128)
    r["biaswA"] = wb6[0:4].astype(bf)
    r["biaswB"] = wb6[4:6].astype(bf)
    return r


def _prep(inputs):
    inputs = {k: np.asarray(v) for k, v in inputs.items()}
    cids = inputs["component_ids"].astype(np.int64)
    skip = inputs["skip_input"].astype(np.int64)
    wid, wlen = skip[..., 0], skip[..., 1]

    # reference's skip reversal
    tt = np.arange(S)[None, :]
    valid = wlen > 0
    rev_pos = np.where(valid, S - tt - wlen, S)
    skip_rev = np.zeros((B, S + 1, 2), np.int64)
    bidx = np.broadcast_to(np.arange(B)[:, None], (B, S))
    skip_rev[bidx, rev_pos] = skip * valid[..., None]
    skip_rev = skip_rev[:, :S]
    cids_r = cids[:, ::-1]
    wid_r, wlen_r = skip_rev[..., 0], skip_rev[..., 1]

    emb_bf = inputs["emb"].astype(bf)
    emb_bf32 = emb_bf.astype(np.float32)   # for exact-cast transposed gathers

    wt = {}
    for d, pre in (("f", "fw_"), ("b", "bw_")):
        a = [inputs[pre + n] for n in
             ["Wih", "Whh", "aWih", "aWhh", "wWih", "wWhh", "b", "ab", "wb"]]
        wt[d] = _weight_tiles(*a)

    sel4 = np.zeros((4, 4 * LD), np.float32)
    for c in range(4):
        sel4[c, c * LD:(c + 1) * LD] = 1.0
    sel2 = np.zeros((2, 2 * LD), np.float32)
    for c in range(2):
        sel2[c, c * LD:(c + 1) * LD] = 1.0

    tag = np.zeros((128, 2, 2, 32), np.float32)
    tw = inputs["tag_W"]          # [512, 32]
    for di in range(2):
        for kc in range(2):
            tag[:, di, kc, :] = tw[256 * di + 128 * kc: 256 * di + 128 * (kc + 1), :]

    shared = {"sel4": sel4.astype(bf), "sel2": sel2.astype(bf),
              "tagw": tag.astype(bf)}
    for d in DIRS:
        for nm in ["xw", "wwih", "whh", "wwhh", "awhh",
                   "biasgA", "biasgB", "biaswA", "biaswB"]:
            shared[f"{nm}_{d}"] = wt[d][nm]

    src = {"f": (cids, wid, wlen), "b": (cids_r, wid_r, wlen_r)}

    in_maps = []
    for k in range(NCORES):
        im = dict(shared)
        for d in DIRS:
            cd, wd, ld = src[d]
            xb, wb2, mm, mw = [], [], [], []
            for j in range(G):
                st = _window_start(G * k + j)
                xb.append(cd[:, st:st + T].T)          # [T, 32]
                wb2.append(wd[:, st:st + T].T)
                m_, h_ = _masks_for_window(ld[:, st:st + T])
                mm.append(m_)
                mw.append(h_)
            xflat = np.concatenate(xb, axis=1).reshape(-1)    # [T*LD]
            wflat = np.concatenate(wb2, axis=1).reshape(-1)
            im[f"x_T_{d}"] = np.ascontiguousarray(
                emb_bf32[xflat, :].T).astype(bf)
            im[f"we_T_{d}"] = np.ascontiguousarray(
                emb_bf32[wflat, :].T).astype(bf)
            mmc = np.concatenate(mm, axis=1).astype(np.uint8)
            mwc = np.concatenate(mw, axis=1).astype(np.uint8)
            im[f"mask_m_{d}"] = np.repeat(mmc[:, None, :], 2,
                                          axis=1).reshape(T, 2 * LD)
            im[f"mask_w_{d}"] = np.repeat(mwc[:, None, :], 2,
                                          axis=1).reshape(T, 2 * LD)
        in_maps.append(im)
    return in_maps


def _postprocess(results, inputs):
    tag_b = np.asarray(inputs["tag_b"])
    out = np.zeros((B, S, L), np.float32)
    for k in range(NCORES):
        arr = results[k]["out_tags"]              # [2, 32, T*LD]
        fwp = arr[0].reshape(L, T, G, 32)         # [label, tau, win, batch]
        bwp = arr[1].reshape(L, T, G, 32)
        for j in range(G):
            w = G * k + j
            st = _window_start(w)
            t0 = _window_t0(w)
            gsl = np.arange(C) + st + t0
            out[:, gsl, :] += fwp[:, t0:t0 + C, j, :].transpose(2, 1, 0)
            out[:, S - 1 - gsl, :] += bwp[:, t0:t0 + C, j, :].transpose(2, 1, 0)
    return out + tag_b[None, None, :]


def _ensure_ntff_hook():
    """The image's antenv lacks axon_hooks; shim it so trace=True works."""
    import sys
    import types
    try:
        from antenv.axon_hooks import get_axon_ntff_profile_hook  # noqa: F401
        return
    except ImportError:
        pass
    import antenv
    from trn_agent_boot.trn_boot import _ntff_profile_via_ctypes
    mod = types.ModuleType("antenv.axon_hooks")
    _state = {"h": _ntff_profile_via_ctypes("/opt/axon/libaxon_pjrt.so")}
    mod.set_axon_ntff_profile_hook = lambda h: _state.__setitem__("h", h)
    mod.get_axon_ntff_profile_hook = lambda: _state["h"]
    sys.modules["antenv.axon_hooks"] = mod
    antenv.axon_hooks = mod


def run(inputs, trace=False):
    if trace:
        _ensure_ntff_hook()
    if "nc" not in _CACHE:
        _CACHE["nc"] = _build_bass()
    nc = _CACHE["nc"]
    in_maps = _prep(inputs)
    res = run_bass_kernel_spmd(nc, in_maps, core_ids=list(range(NCORES)),
                               trace=trace)
    out = _postprocess(res.results, {k: np.asarray(v) for k, v in inputs.items()})
    return out, res


def kernel(**inputs):
    out, _ = run(inputs, trace=False)
    return out
